# revision 9
# baseline (speedup 1.0000x reference)
"""Trainium2 Bass kernel for nn_DualAddressingPhasor.

Math: the phasor cumsum-bind/retrieve is causal linear attention:
  retrieved[l] = sum_{l'<=l} (sum_k cos(phi_l,k - phi_l',k)) * value[l']
Per 512-row chunk this is (1) a carried state [2K, D] = CS^T @ value over
the prefix plus (2) intra-chunk attention triu(Cc@Cc^T + Sc@Sc^T) @ value_c.

Sharding: 8 cores = 2 batches x 4 sequence chunks of 512. Uniform SPMD
program; per-core variation is entirely in the data (right-aligned
zero-padded prefix, host-precomputed positional sin/cos with zeros in the
padding so padded rows contribute nothing).
"""

import sys

for _p in ("/opt/trn_rl_repo",):
    if _p not in sys.path:
        sys.path.append(_p)

import numpy as np

import concourse.bacc as bacc
import concourse.tile as tile
import concourse.mybir as mybir
from concourse.bass import ts
from concourse.bass_utils import run_bass_kernel_spmd
from concourse.masks import make_identity

F32 = mybir.dt.float32
F32R = mybir.dt.float32r
AF = mybir.ActivationFunctionType
ALU = mybir.AluOpType

D = 512
K = 32
B = 2
L = 2048
CH = 512          # chunk rows per core
T = 2048          # padded rows processed per core
NCORE = 8
HALF_PI = float(np.pi / 2)

_NC_CACHE = {}
LAST_RESULT = None
RUN_KWARGS = {}


def _build():
    nc = bacc.Bacc("TRN2", num_devices=NCORE)

    xt = nc.dram_tensor("xt", [4, 128, T], F32R, kind="ExternalInput")
    w1 = nc.dram_tensor("w1", [4, 128, D], F32R, kind="ExternalInput")
    w2 = nc.dram_tensor("w2", [4, 128, K], F32R, kind="ExternalInput")
    wv = nc.dram_tensor("wv", [4, 128, D], F32R, kind="ExternalInput")
    wo = nc.dram_tensor("wo", [4, 128, D], F32R, kind="ExternalInput")
    ncs = nc.dram_tensor("ncs", [1, D], F32R, kind="ExternalInput")
    bvr = nc.dram_tensor("bvr", [1, D], F32R, kind="ExternalInput")
    b1p = nc.dram_tensor("b1p", [128, 4], F32, kind="ExternalInput")
    b2p = nc.dram_tensor("b2p", [128, 1], F32, kind="ExternalInput")
    pcos = nc.dram_tensor("pcos", [128, CH], F32, kind="ExternalInput")
    psin = nc.dram_tensor("psin", [128, CH], F32, kind="ExternalInput")
    res = nc.dram_tensor("res", [4, 128, D], F32, kind="ExternalInput")
    y = nc.dram_tensor("y", [CH, D], F32, kind="ExternalOutput")

    kc = nc.dram_tensor("kc", [128, 1], F32, kind="ExternalInput")  # pi*content_scale

    with tile.TileContext(nc) as tc:
        with (
            tc.tile_pool(name="const", bufs=1) as cp_,
            tc.tile_pool(name="big", bufs=1) as bigp,
            tc.tile_pool(name="rot", bufs=2) as rot,
            tc.tile_pool(name="pmm", bufs=2, space="PSUM") as pmm,
            tc.tile_pool(name="pone", bufs=1, space="PSUM") as pone,
            tc.tile_pool(name="ptr", bufs=2, space="PSUM") as ptrp,
        ):
            # ---- constant / input loads ----
            xt_sb = bigp.tile([128, 4, T], F32R)
            for k in range(4):
                nc.sync.dma_start(xt_sb[:, k, :], xt[k])
            w1_sb = cp_.tile([128, 4, D], F32R)
            nc.sync.dma_start(w1_sb[:], w1[:].transpose([1, 0, 2]))
            w2_sb = cp_.tile([128, 4, K], F32R)
            nc.sync.dma_start(w2_sb[:], w2[:].transpose([1, 0, 2]))
            wv_sb = cp_.tile([128, 4, D], F32R)
            nc.sync.dma_start(wv_sb[:], wv[:].transpose([1, 0, 2]))
            wo_sb = cp_.tile([128, 4, D], F32R)
            nc.sync.dma_start(wo_sb[:], wo[:].transpose([1, 0, 2]))
            ncs_sb = cp_.tile([1, D], F32R)
            nc.sync.dma_start(ncs_sb[:], ncs[:])
            bvr_sb = cp_.tile([1, D], F32R)
            nc.sync.dma_start(bvr_sb[:], bvr[:])
            b1p_sb = cp_.tile([128, 4], F32)
            nc.sync.dma_start(b1p_sb[:], b1p[:])
            b2p_sb = cp_.tile([128, 1], F32)
            nc.sync.dma_start(b2p_sb[:], b2p[:])
            pcos_sb = cp_.tile([128, CH], F32)
            nc.sync.dma_start(pcos_sb[:], pcos[:])
            psin_sb = cp_.tile([128, CH], F32)
            nc.sync.dma_start(psin_sb[:], psin[:])
            res_sb = cp_.tile([128, 4, D], F32)
            nc.sync.dma_start(res_sb[:], res[:].transpose([1, 0, 2]))
            kc_sb = cp_.tile([128, 1], F32)
            nc.sync.dma_start(kc_sb[:], kc[:])

            onesf = cp_.tile([128, 128], F32)
            nc.vector.memset(onesf[:], 1.0)
            onesr = cp_.tile([1, 128], F32R)
            nc.vector.tensor_copy(onesr[:], onesf[0:1, :])
            onesc = cp_.tile([128, 1], F32R)
            nc.vector.tensor_copy(onesc[:], onesf[:, 0:1])
            halfpi = cp_.tile([128, 1], F32)
            nc.vector.memset(halfpi[:], HALF_PI)
            epsb = cp_.tile([128, 1], F32)
            nc.vector.memset(epsb[:], 1e-5)

            identf = cp_.tile([128, 128], F32)
            make_identity(nc, identf[:])
            identr = cp_.tile([128, 128], F32R)
            nc.vector.tensor_copy(identr[:], identf[:])

            # triangular masks for intra-chunk causal attention (lhsT form:
            # tri[p, tr, y] = 1 iff y >= p + 128*tr)
            tri = cp_.tile([128, 4, CH], F32)
            for tr in range(4):
                nc.gpsimd.memset(tri[:, tr, :], 0.0)
                nc.gpsimd.affine_select(
                    out=tri[:, tr, :], in_=tri[:, tr, :],
                    compare_op=ALU.is_gt, fill=1.0, base=128 * tr,
                    pattern=[[-1, CH]], channel_multiplier=1,
                )

            # ---- (a) h^T = tanh(W1^T x^T + b1), chunked; (b) packed content phase ----
            tt_sb = cp_.tile([128, CH], F32)
            for c in range(4):
                h_ck = rot.tile([128, 4, CH], F32R, tag="hck")
                for dout in range(4):
                    ph = pmm.tile([128, CH], F32, tag="pmm")
                    for k in range(4):
                        nc.tensor.matmul(
                            ph[:], w1_sb[:, k, ts(dout, 128)],
                            xt_sb[:, k, ts(c, CH)],
                            start=(k == 0), stop=(k == 3),
                        )
                    nc.scalar.activation(
                        h_ck[:, dout, :], ph[:], AF.Tanh,
                        bias=b1p_sb[:, dout : dout + 1], scale=1.0,
                    )
                pc = pmm.tile([32, CH], F32, tag="pmm")
                for k in range(4):
                    nc.tensor.matmul(
                        pc[:], w2_sb[:, k, :], h_ck[:, k, :],
                        start=(k == 0), stop=(k == 3),
                    )
                nc.scalar.activation(
                    tt_sb[32 * c : 32 * c + 32, :], pc[:], AF.Tanh,
                    bias=b2p_sb[0:32, :], scale=1.0,
                )

            # ---- (c) value = x @ Wv + bv (natural layout) ----
            value_sb = bigp.tile([128, 16, D], F32R)
            for tt in range(16):
                pv = pmm.tile([128, D], F32, tag="pmm")
                for k in range(4):
                    nc.tensor.matmul(
                        pv[:], xt_sb[:, k, ts(tt, 128)], wv_sb[:, k, :],
                        start=(k == 0), stop=False,
                    )
                nc.tensor.matmul(pv[:], onesr[:], bvr_sb[:], start=False, stop=True)
                nc.vector.tensor_copy(value_sb[:, tt, :], pv[:])

            # ---- phases: C/S = exp(i(pos + content)) via angle addition ----
            sct = cp_.tile([128, CH], F32)
            nc.scalar.activation(sct[:], tt_sb[:], AF.Sin, scale=kc_sb[:])
            cct = cp_.tile([128, CH], F32)
            nc.scalar.activation(cct[:], tt_sb[:], AF.Sin, bias=halfpi[:], scale=kc_sb[:])
            mm1 = cp_.tile([128, CH], F32)
            nc.vector.tensor_mul(mm1[:], pcos_sb[:], cct[:])
            mm2 = cp_.tile([128, CH], F32)
            nc.vector.tensor_mul(mm2[:], psin_sb[:], sct[:])
            mm3 = cp_.tile([128, CH], F32)
            nc.vector.tensor_mul(mm3[:], psin_sb[:], cct[:])
            mm4 = cp_.tile([128, CH], F32)
            nc.vector.tensor_mul(mm4[:], pcos_sb[:], sct[:])
            cpk = cp_.tile([128, CH], F32R)
            nc.vector.tensor_sub(cpk[:], mm1[:], mm2[:])
            spk = cp_.tile([128, CH], F32R)
            nc.vector.tensor_add(spk[:], mm3[:], mm4[:])

            # ---- transpose C/S to natural layout for the state matmul ----
            # csm[p, c, b, j]: natural row l = 512c + 128b + p, freq j (0:32 C, 32:64 S)
            csm = cp_.tile([128, 4, 4, 64], F32R)
            for src, j0 in ((cpk, 0), (spk, 32)):
                for bb in range(4):
                    ptr = ptrp.tile([128, 128], F32R, tag="ptr")
                    nc.tensor.transpose(ptr[:], src[:, ts(bb, 128)], identr[:])
                    nc.vector.tensor_copy(
                        csm[:, :, bb, j0 : j0 + 32],
                        ptr[:].rearrange("p (c j) -> p c j", j=32),
                    )

            # ---- (d) prefix state [2K, D]; moved to partitions 96:128 for (g) ----
            pst = pone.tile([32, 1024], F32, tag="pst")
            for part, j0 in ((0, 0), (1, 32)):
                for kt in range(12):
                    c, bb = kt // 4, kt % 4
                    nc.tensor.matmul(
                        pst[0:32, ts(part, D)],
                        csm[:, c, bb, j0 : j0 + 32],
                        value_sb[:, kt, :],
                        start=(kt == 0), stop=(kt == 11),
                    )
            state_sb = cp_.tile([128, 1024], F32R)
            nc.vector.tensor_copy(state_sb[96:128, :], pst[0:32, :])

            # ---- (e) intra-chunk scores, triu-masked ----
            p_sb = cp_.tile([128, 4, CH], F32R)
            for tr in range(4):
                psc = pmm.tile([128, CH], F32, tag="pmm")
                nc.tensor.matmul(
                    psc[:], cpk[96:128, ts(tr, 128)], cpk[96:128, :],
                    start=True, stop=False, tile_position=(96, 0),
                )
                nc.tensor.matmul(
                    psc[:], spk[96:128, ts(tr, 128)], spk[96:128, :],
                    start=False, stop=True, tile_position=(96, 0),
                )
                nc.vector.tensor_mul(p_sb[:, tr, :], psc[:], tri[:, tr, :])

            # ---- (f)+(g) retrieved^T [D, CH] ----
            retrT = cp_.tile([128, 4, CH], F32R)
            sq_sb = cp_.tile([128, 4, CH], F32R)
            for dd in range(4):
                pr = pmm.tile([128, CH], F32, tag="pmm")
                for tr in range(4):
                    nc.tensor.matmul(
                        pr[:], value_sb[:, 12 + tr, ts(dd, 128)], p_sb[:, tr, :],
                        start=(tr == 0), stop=False,
                    )
                nc.tensor.matmul(
                    pr[:], state_sb[96:128, ts(dd, 128)], cpk[96:128, :],
                    start=False, stop=False, tile_position=(96, 0),
                )
                nc.tensor.matmul(
                    pr[:], state_sb[96:128, 512 + 128 * dd : 512 + 128 * dd + 128],
                    spk[96:128, :],
                    start=False, stop=True, tile_position=(96, 0),
                )
                nc.vector.tensor_copy(retrT[:, dd, :], pr[:])
                nc.scalar.square(sq_sb[:, dd, :], pr[:])

            # ---- LayerNorm stats (feature dim = partitions here) ----
            ps_stat = pone.tile([1, 2 * CH], F32, tag="pb")
            for dd in range(4):
                nc.tensor.matmul(
                    ps_stat[0:1, 0:CH], onesc[:], retrT[:, dd, :],
                    start=(dd == 0), stop=(dd == 3),
                )
            for dd in range(4):
                nc.tensor.matmul(
                    ps_stat[0:1, CH : 2 * CH], onesc[:], sq_sb[:, dd, :],
                    start=(dd == 0), stop=(dd == 3),
                )
            mu_n = cp_.tile([1, CH], F32R)
            nc.vector.tensor_scalar_mul(mu_n[:], ps_stat[0:1, 0:CH], 1.0 / D)
            stats2 = cp_.tile([1, 2 * CH], F32)
            nc.vector.tensor_copy(stats2[:], ps_stat[:])

            # transpose stats to [128, 4, 2] for per-partition rstd
            statsT = cp_.tile([128, 4, 2], F32)
            for s in range(2):
                for tq in range(4):
                    ptr2 = ptrp.tile([128, 128], F32, tag="ptr")
                    nc.tensor.transpose(
                        ptr2[:, 0:1], stats2[0:1, CH * s + 128 * tq : CH * s + 128 * tq + 128],
                        identf[0:1, 0:1],
                    )
                    nc.vector.tensor_copy(statsT[:, tq, s : s + 1], ptr2[:, 0:1])
            muT = cp_.tile([128, 4], F32)
            nc.vector.tensor_scalar_mul(muT[:], statsT[:, :, 0], 1.0 / D)
            varT = cp_.tile([128, 4], F32)
            nc.vector.tensor_scalar_mul(varT[:], statsT[:, :, 1], 1.0 / D)
            mu2T = cp_.tile([128, 4], F32)
            nc.vector.tensor_mul(mu2T[:], muT[:], muT[:])
            nc.vector.tensor_sub(varT[:], varT[:], mu2T[:])
            sdT = cp_.tile([128, 4], F32)
            nc.scalar.activation(sdT[:], varT[:], AF.Sqrt, bias=epsb[:], scale=1.0)
            rstdT = cp_.tile([128, 4], F32)
            nc.vector.reciprocal(rstdT[:], sdT[:])

            # ---- (h) out = rstd*(center^T @ Wo') + res ----
            for tt in range(4):
                pho = pmm.tile([128, D], F32, tag="pmm")
                for ee in range(4):
                    nc.tensor.matmul(
                        pho[:], retrT[:, ee, ts(tt, 128)], wo_sb[:, ee, :],
                        start=(ee == 0), stop=False,
                    )
                nc.tensor.matmul(
                    pho[:], mu_n[0:1, ts(tt, 128)], ncs_sb[:],
                    start=False, stop=True,
                )
                out_t = rot.tile([128, D], F32, tag="outt")
                nc.vector.scalar_tensor_tensor(
                    out=out_t[:], in0=pho[:], scalar=rstdT[:, tt : tt + 1],
                    in1=res_sb[:, tt, :], op0=ALU.mult, op1=ALU.add,
                )
                nc.sync.dma_start(y[ts(tt, 128), :], out_t[:])

    nc.compile()
    return nc


def _get_nc():
    if "nc" not in _NC_CACHE:
        _NC_CACHE["nc"] = _build()
    return _NC_CACHE["nc"]


def _prep_inputs(inputs):
    x = np.asarray(inputs["x"], np.float32)
    W1 = np.asarray(inputs["W1"], np.float32)
    b1 = np.asarray(inputs["b1"], np.float32)
    W2 = np.asarray(inputs["W2"], np.float32)
    b2 = np.asarray(inputs["b2"], np.float32)
    pos_scale = float(np.asarray(inputs["pos_scale"]).reshape(-1)[0])
    content_scale = float(np.asarray(inputs["content_scale"]).reshape(-1)[0])
    Wv = np.asarray(inputs["Wv"], np.float32)
    bv = np.asarray(inputs["bv"], np.float32)
    ln_g = np.asarray(inputs["ln_g"], np.float32)
    ln_b = np.asarray(inputs["ln_b"], np.float32)
    Wo = np.asarray(inputs["Wo"], np.float32)
    bo = np.asarray(inputs["bo"], np.float32)

    Wop = ln_g[:, None] * Wo                       # fold ln gain
    ncs_v = -Wop.sum(axis=0, dtype=np.float64).astype(np.float32)[None, :]
    res_base = (ln_b @ Wo + bo).astype(np.float32)  # fold ln bias + out bias

    w1_t = np.ascontiguousarray(W1.reshape(4, 128, D))
    w2_t = np.ascontiguousarray(W2.reshape(4, 128, K))
    wv_t = np.ascontiguousarray(Wv.reshape(4, 128, D))
    wo_t = np.ascontiguousarray(Wop.reshape(4, 128, D))
    b1p = np.ascontiguousarray(b1.reshape(4, 128).T)
    b2p = np.tile(b2, 4)[:, None].astype(np.float32)
    kc = np.full((128, 1), np.pi * content_scale, np.float32)
    bvr = bv[None, :].astype(np.float32)

    freqs = 1.0 / (10000.0 ** (np.arange(K, dtype=np.float64) / K))

    in_maps = []
    for core in range(NCORE):
        b, i = divmod(core, 4)
        pad = 1536 - 512 * i
        nreal = 512 * (i + 1)
        xpad = np.zeros((T, D), np.float32)
        xpad[pad:] = x[b, :nreal]
        # xt dram layout: [k, 128, T] where row = Din 128k+p, col = padded row r
        xt = np.ascontiguousarray(xpad.T.reshape(4, 128, T))

        lidx = np.arange(T, dtype=np.float64) - pad
        ang = pos_scale * lidx[:, None] * freqs[None, :]      # [T, K]
        pc = np.cos(ang).astype(np.float32)
        psn = np.sin(ang).astype(np.float32)
        pc[lidx < 0] = 0.0
        psn[lidx < 0] = 0.0
        pcos = np.ascontiguousarray(
            pc.reshape(4, CH, K).transpose(0, 2, 1).reshape(128, CH))
        psin = np.ascontiguousarray(
            psn.reshape(4, CH, K).transpose(0, 2, 1).reshape(128, CH))

        resc = (x[b, 512 * i : 512 * i + CH] + res_base[None, :]).astype(np.float32)

        in_maps.append({
            "xt": xt, "w1": w1_t, "w2": w2_t, "wv": wv_t, "wo": wo_t,
            "ncs": ncs_v, "bvr": bvr, "b1p": b1p, "b2p": b2p,
            "pcos": pcos, "psin": psin,
            "res": np.ascontiguousarray(resc.reshape(4, 128, D)),
            "kc": kc,
        })
    return in_maps


def kernel(**inputs) -> np.ndarray:
    global LAST_RESULT
    nc = _get_nc()
    in_maps = _prep_inputs(inputs)
    result = run_bass_kernel_spmd(
        nc, in_maps, core_ids=list(range(NCORE)), **RUN_KWARGS
    )
    LAST_RESULT = result
    y = np.empty((B, L, D), np.float32)
    for core in range(NCORE):
        b, i = divmod(core, 4)
        y[b, 512 * i : 512 * i + CH] = result.results[core]["y"]
    return y
